# revision 1
# baseline (speedup 1.0000x reference)
"""Trainium2 Bass kernel for ConvReshapeBefore (im2col patch extraction).

Full problem: x (32, 64, 64, 64) f32 NHWC, kernel 3x3 stride 1 valid ->
out (62*62*32, 3, 3, 64) f32 where out[(r*62+c)*32 + b] = x[b, r:r+3, c:c+3, :].

Sharding: data-parallel over batch, 4 batches per core across 8 cores.

Pure-DMA design: the 9x window replication is expressed as overlapping
DMA reads from SBUF, so no compute engine touches the data at all.

  1. load x shard into SBUF interleaved: partition p = 2*h + (b//2),
     free = (b%2)*4096 + w*64 + k.  Even partitions hold batches 0-1,
     odd partitions hold batches 2-3 -> every store below spans both
     bit-6 partition halves and therefore all 16 SBUF AXI ports.
  2. 12 HWDGE stores, one per (b, i): 62 stride-2 partitions x 62
     overlapping c-windows x 192 f32 (j,k) contiguous runs:
       src [[2*8192, 62], [64, 62], [1, 192]] @ part 2i+b//2
       dst [[142848, 62], [2304, 62], [1, 192]] @ b*576 + i*192
     2.95 MB per store, 768 B descriptors.  Batches 0-1 issue on the
     SP HWDGE ring, batches 2-3 on the ACT ring, so the two rings'
     descriptor generation runs concurrently.
"""

import numpy as np

import concourse.bass as bass
import concourse.mybir as mybir
from concourse.ap import AP
from concourse.bass_utils import run_bass_kernel_spmd

# Full-problem constants (hardcoded per harness contract)
B, H, W, C = 32, 64, 64, 64
K = 3
R = H - K + 1  # 62
NCORES = 8
BS = B // NCORES  # 4

PITCH = 2 * W * C          # 8192 f32 per partition (2 batches)
BHWC = H * W * C           # 262144 f32 per batch in x
JK = K * C                 # 192 f32 contiguous (j, k) run
OUT_C = BS * K * K * C     # 2304 f32 per (r, c)
OUT_R = R * OUT_C          # 142848 f32 per r


def _build_nc() -> bass.Bass:
    nc = bass.Bass(target_bir_lowering=False)
    x = nc.dram_tensor("x", [BS, H, W, C], mybir.dt.float32, kind="ExternalInput")
    out = nc.dram_tensor(
        "out", [R * R * BS, K, K, C], mybir.dt.float32, kind="ExternalOutput"
    )

    def load_aps(e):
        # batches 2e, 2e+1 -> partitions 2h+e
        dst = AP(xt, e * PITCH, [[2 * PITCH, H], [W * C, 2], [1, W * C]])
        src = AP(x, 2 * e * BHWC, [[W * C, H], [BHWC, 2], [1, W * C]])
        return dst, src

    # r-blocks keep each SWDGE dma_start under ~1000 descriptor-pairs: one
    # call's descriptors must fit the Q7 descriptor rings (62 rows = 3844
    # pairs deadlocks the device, and even 31 rows = 1922 pairs does; 16
    # rows = 992 pairs is measured-safe).
    RBLK = [(r0, min(16, R - r0)) for r0 in range(0, R, 16)]

    def store_aps(b, i, r0, rn):
        e, bl = b // 2, b % 2
        src = AP(
            xt,
            (2 * (i + r0) + e) * PITCH + bl * W * C,
            [[2 * PITCH, rn], [C, R], [1, JK]],
        )
        dst = AP(
            out,
            r0 * OUT_R + b * K * JK + i * JK,
            [[OUT_R, rn], [OUT_C, R], [1, JK]],
        )
        return dst, src

    with (
        nc.sbuf_tensor("xt", [128, PITCH], mybir.dt.float32) as xt,
        nc.semaphore("l0") as l0,
        nc.semaphore("l1") as l1,
        nc.semaphore("s0") as s0,
        nc.semaphore("s1") as s1,
        nc.Block() as block,
    ):
        @block.gpsimd
        def _(gp):
            # All DMAs via SWDGE: store descriptors spread across all 16
            # SDMA engines (HWDGE pins DRAM-destination DMAs to one engine
            # per queue), and SWDGE descriptor emission (~0.34 ns/desc) is
            # far faster than HWDGE generation for the loads too.  Loads
            # are emitted first; stores keep the full-barrier waits and the
            # measured-fastest i-major issue order.
            dst, src = load_aps(0)
            gp.dma_start(dst, src).then_inc(l0, 16)
            dst, src = load_aps(1)
            gp.dma_start(dst, src).then_inc(l1, 16)
            gp.wait_ge(l0, 16)
            n0 = n1 = 0
            for i in range(K):
                for b in (0, 1):
                    for r0, rn in RBLK:
                        dst, src = store_aps(b, i, r0, rn)
                        gp.dma_start(dst, src).then_inc(s0, 16)
                        n0 += 1
            gp.wait_ge(l1, 16)
            for i in range(K):
                for b in (2, 3):
                    for r0, rn in RBLK:
                        dst, src = store_aps(b, i, r0, rn)
                        gp.dma_start(dst, src).then_inc(s1, 16)
                        n1 += 1
            gp.wait_ge(s0, 16 * n0)
            gp.wait_ge(s1, 16 * n1)

    return nc


_NC = None


def _get_nc():
    global _NC
    if _NC is None:
        _NC = _build_nc()
    return _NC


def kernel(x: np.ndarray, **_run_kwargs) -> np.ndarray:
    assert x.shape == (B, H, W, C), x.shape
    nc = _get_nc()
    x = np.ascontiguousarray(x, dtype=np.float32)
    in_maps = [{"x": x[d * BS : (d + 1) * BS]} for d in range(NCORES)]
    res = run_bass_kernel_spmd(nc, in_maps, list(range(NCORES)), **_run_kwargs)
    outs = [res.results[d]["out"].reshape(R * R, BS, K, K, C) for d in range(NCORES)]
    full = np.concatenate(outs, axis=1).reshape(R * R * B, K, K, C)
    if _run_kwargs:
        return full, res
    return full

